# revision 21
# baseline (speedup 1.0000x reference)
"""BlockDWT2d (3-level Haar wavelet packet, 8x8 blocks) on 8 TRN2 NeuronCores.

Input  x: (32, 3, 512, 512) fp32 -> output (32, 192, 64, 64) fp32.

Math: the 3-level full packet transform is separable: for each 8x8 input
block, out2d = (H8/8) @ X8 @ H8^T where H8 is the natural-binary-order
Hadamard matrix; output channel k_sub bit-interleaves the row/col
transform indices (kH, kW): k_sub = 32h2+16w2+8h1+4w1+2h0+w0, and the
full channel index is K = 3*k_sub + c.

Per-core pipeline (batch-sharded 4 samples/core, 12 images of 512x512):
  DMA in:  X [p=h(128) x4 tiles, f=w(512)] (contiguous, GPSIMD SWDGE)
  Pass A   (per q: 4 mms t=0..3): psA[:, 128t:] = X[:,t,128q:]^T @ BD1
           BD1[(g,i),(kH,g')] = H8[kH,i]/8 * delta_gg'
           -> [p=(xbl,j), f=(t,kH,g)]; one copy/regroup -> Bq[p,(kH,yb)]
  Pass B   (per r: 4 mms q=0..3): psB[:, 128q:] = Bq[:, 128r:]^T @ BD2
           BD2[(xbl,j),(kW,xbl')] = H8[kW,j] * delta_xbl
           -> [p=(h0,yb), f=(q,kW,xbl)]; one copy/regroup -> Fr[p,(kW,xb)]
  DMA out: direct HWDGE DMAs, one per (img, r, kW): src [128p, 64f],
           DRAM dims (h0, yb, xb) — 3-dim AP, 256B runs. Alternated
           between the SP and ACT HWDGE rings.
"""

import numpy as np

_CACHE = {}


def _h8():
    x = np.eye(8, dtype=np.float32).reshape(1, 8, 8)
    for _ in range(3):
        a, b = x[:, 0::2, :], x[:, 1::2, :]
        x = np.concatenate([a + b, a - b], axis=0)
    return x[:, 0, :]  # H8[k, i], entries +-1


def _interleave(kH, kW):
    h2, h1, h0 = (kH >> 2) & 1, (kH >> 1) & 1, kH & 1
    w2, w1, w0 = (kW >> 2) & 1, (kW >> 1) & 1, kW & 1
    return 32 * h2 + 16 * w2 + 8 * h1 + 4 * w1 + 2 * h0 + 1 * w0


def _constants():
    H8 = _h8()
    bd1 = np.zeros((128, 128), np.float32)  # [(g,i), (kH,g')]
    for g in range(16):
        bd1[g * 8:(g + 1) * 8, :].reshape(8, 8, 16)[:, :, g] = (H8.T / 8.0)
    bd2 = np.zeros((128, 128), np.float32)  # [(xbl,j), (kW,xbl')]
    for xbl in range(16):
        bd2[xbl * 8:(xbl + 1) * 8, :].reshape(8, 8, 16)[:, :, xbl] = H8.T
    return {"bd1": bd1, "bd2": bd2}


def _build_body(nc, variant, x_in, ov, bd1_s, bd2_s,
                xpool, bpool, fpool, ppool, mybir, rep=0):
    do_mm = variant not in ("dmaonly",)
    do_out = variant not in ("noout",)
    ndma = 0
    for img in range(12):
        xt = xpool.tile([128, 4, 512], mybir.dt.float32, tag="x",
                        name=f"x_{rep}_{img}")
        nc.gpsimd.dma_start(
            xt[:], x_in.ap()[img].rearrange("(t p) w -> p t w", p=128))

        bqs = []
        if do_mm:
            for q in range(4):
                psa = ppool.tile([128, 512], mybir.dt.float32, tag="ps",
                                 name=f"psA_{rep}_{img}_{q}")
                for t in range(4):
                    nc.tensor.matmul(psa[:, t * 128:(t + 1) * 128],
                                     lhsT=xt[:, t, q * 128:(q + 1) * 128],
                                     rhs=bd1_s[:], start=True, stop=True)
                bq = bpool.tile([128, 512], mybir.dt.float32, tag="bq",
                                name=f"bq_{rep}_{img}_{q}")
                dst = bq.rearrange("p (a t g) -> p t a g", a=8, t=4)
                src = psa.rearrange("p (t a g) -> p t a g", t=4, a=8)
                nc.vector.tensor_copy(dst, src)
                bqs.append(bq)

        b, c = img // 3, img % 3
        for r in range(4):
            h2, h1 = r // 2, r % 2
            fr = fpool.tile([128, 512], mybir.dt.float32, tag="fr",
                            name=f"fr_{rep}_{img}_{r}")
            if do_mm:
                psb = ppool.tile([128, 512], mybir.dt.float32, tag="ps",
                                 name=f"psB_{rep}_{img}_{r}")
                for q in range(4):
                    nc.tensor.matmul(psb[:, q * 128:(q + 1) * 128],
                                     lhsT=bqs[q][:, r * 128:(r + 1) * 128],
                                     rhs=bd2_s[:], start=True, stop=True)
                dst = fr.rearrange("p (a q g) -> p q a g", a=8, q=4)
                src = psb.rearrange("p (q a g) -> p q a g", q=4, a=8)
                nc.vector.tensor_copy(dst, src)
            else:
                nc.vector.tensor_copy(fr[:], xt[:, r, :])

            if do_out:
                for kw in range(8):
                    w2, w1, w0 = kw // 4, (kw // 2) % 2, kw % 2
                    # dst dims (h0, yb, xb) matches src enumeration
                    dma_dst = ov[b, h2, w2, h1, w1, :, w0, c, :, :]
                    eng = nc.sync if ndma % 2 == 0 else nc.scalar
                    eng.dma_start(dma_dst, fr[:, kw * 64:(kw + 1) * 64])
                    ndma += 1


def _build_nc(variant="full"):
    from contextlib import ExitStack

    import concourse.tile as tile
    from concourse import bacc, mybir

    nc = bacc.Bacc("TRN2", target_bir_lowering=False, debug=False)

    x_in = nc.dram_tensor("x", [12, 512, 512], mybir.dt.float32,
                          kind="ExternalInput")
    bd1_d = nc.dram_tensor("bd1", [128, 128], mybir.dt.float32,
                           kind="ExternalInput")
    bd2_d = nc.dram_tensor("bd2", [128, 128], mybir.dt.float32,
                           kind="ExternalInput")
    out_d = nc.dram_tensor("out", [4, 192, 64, 64], mybir.dt.float32,
                           kind="ExternalOutput")
    # out view: [b, h2, w2, h1, w1, h0, w0, c, yb, xb]
    ov = out_d.ap().rearrange(
        "bb (h2 w2 h1 w1 h0 w0 c) yb xb -> bb h2 w2 h1 w1 h0 w0 c yb xb",
        h2=2, w2=2, h1=2, w1=2, h0=2, w0=2, c=3)

    with tile.TileContext(nc) as tc, ExitStack() as ctx:
        cpool = ctx.enter_context(tc.tile_pool(name="consts", bufs=1))
        xpool = ctx.enter_context(tc.tile_pool(name="xin", bufs=2))
        bpool = ctx.enter_context(tc.tile_pool(name="bq", bufs=9))
        fpool = ctx.enter_context(tc.tile_pool(name="fr", bufs=9))
        ppool = ctx.enter_context(tc.tile_pool(name="ps", bufs=6, space="PSUM"))

        bd1_s = cpool.tile([128, 128], mybir.dt.float32, tag="bd1")
        bd2_s = cpool.tile([128, 128], mybir.dt.float32, tag="bd2")
        nc.gpsimd.dma_start(bd1_s[:], bd1_d.ap())
        nc.gpsimd.dma_start(bd2_s[:], bd2_d.ap())

        reps = 2 if variant == "double" else 1
        for rep in range(reps):
            _build_body(nc, variant, x_in, ov, bd1_s, bd2_s,
                        xpool, bpool, fpool, ppool, mybir, rep=rep)

    nc.compile()
    return nc


def _get_nc(variant="full"):
    if variant not in _CACHE:
        _CACHE[variant] = _build_nc(variant)
    return _CACHE[variant]


def kernel(x: np.ndarray) -> np.ndarray:
    from concourse.bass_utils import run_bass_kernel_spmd

    assert x.shape == (32, 3, 512, 512) and x.dtype == np.float32
    nc = _get_nc()
    consts = _constants()
    in_maps = []
    for i in range(8):
        shard = np.ascontiguousarray(
            x[4 * i:4 * i + 4].reshape(12, 512, 512))
        in_maps.append({"x": shard, **consts})
    res = run_bass_kernel_spmd(nc, in_maps, core_ids=list(range(8)))
    return np.concatenate([r["out"] for r in res.results], axis=0)


# revision 37
# speedup vs baseline: 12.2738x; 12.2738x over previous
"""BlockDWT2d (3-level Haar wavelet packet, 8x8 blocks) on 8 TRN2 NeuronCores.

Input  x: (32, 3, 512, 512) fp32 -> output (32, 192, 64, 64) fp32.

Math: the 3-level full packet transform is separable: for each 8x8 input
block, out2d = (H8/8) @ X8 @ H8^T where H8 is the natural-binary-order
Hadamard matrix; output channel k_sub bit-interleaves the row/col
transform indices (kH, kW): k_sub = 32h2+16w2+8h1+4w1+2h0+w0, and the
full channel index is K = 3*k_sub + c.

Per-core pipeline (batch-sharded 4 samples/core, 12 images of 512x512):
  DMA in:  X [p=h(128) x4 tiles, f=w(512)] (contiguous, GPSIMD SWDGE)
  Pass A   (per q: 4 mms t=0..3): psA[:, 128t:] = X[:,t,128q:]^T @ BD1
           BD1[(g,i),(kH,g')] = H8[kH,i]/8 * delta_gg'
           -> [p=(xbl,j), f=(t,kH,g)]; one copy/regroup -> Bq[p,(kH,yb)]
  Pass B   (per r: 4 mms q=0..3): psB[:, 128q:] = Bq[:, 128r:]^T @ BD2
           BD2[(xbl,j),(kW,xbl')] = H8[kW,j] * delta_xbl
           -> [p=(h0,yb), f=(q,kW,xbl)]; one copy/regroup -> Fr[p,(kW,xb)]
  DMA out: direct HWDGE DMAs, one per (img, r, kW): src [128p, 64f],
           DRAM dims (h0, yb, xb) — 3-dim AP, 256B runs. Alternated
           between the SP and ACT HWDGE rings.
"""

import numpy as np

_CACHE = {}


def _h8():
    x = np.eye(8, dtype=np.float32).reshape(1, 8, 8)
    for _ in range(3):
        a, b = x[:, 0::2, :], x[:, 1::2, :]
        x = np.concatenate([a + b, a - b], axis=0)
    return x[:, 0, :]  # H8[k, i], entries +-1


def _interleave(kH, kW):
    h2, h1, h0 = (kH >> 2) & 1, (kH >> 1) & 1, kH & 1
    w2, w1, w0 = (kW >> 2) & 1, (kW >> 1) & 1, kW & 1
    return 32 * h2 + 16 * w2 + 8 * h1 + 4 * w1 + 2 * h0 + 1 * w0


def _constants():
    H8 = _h8()
    bd1 = np.zeros((128, 128), np.float32)  # [(g,i), (kH,g')]
    for g in range(16):
        bd1[g * 8:(g + 1) * 8, :].reshape(8, 8, 16)[:, :, g] = (H8.T / 8.0)
    bd2 = np.zeros((128, 128), np.float32)  # [(xbl,j), (kW,xbl')]
    for xbl in range(16):
        bd2[xbl * 8:(xbl + 1) * 8, :].reshape(8, 8, 16)[:, :, xbl] = H8.T
    return {"bd1": bd1, "bd2": bd2}


def _build_body_v4(nc, variant, x_in, out_v, bd1_s, bd2_s,
                   xpool, bpool, fpool, ppool, ppoolb, mybir, rep=0):
    """Pass-B M=64 (p=yb only); stage whole K-halves; 2 out-DMAs per sample.

    out_v: [4, 192, 64, 64] AP (ExternalOutput or scratch).
    Staging S_{b,h2} [64p(yb), f=(Klocal 96, xb 64)]; DMA dims
    [yb][K][xb]. Copy dst via 8-dim rearrange view.
    """
    do_out = "noout" not in variant
    in_eng = nc.sync if "insync" in variant else nc.gpsimd
    ncopy = 0
    for b in range(4):
        bqs_c = []
        for c in range(3):
            xt = xpool.tile([128, 4, 512], mybir.dt.float32, tag="x",
                            name=f"x_{rep}_{b}_{c}")
            in_eng.dma_start(
                xt[:], x_in.ap()[b * 3 + c].rearrange("(t p) w -> p t w",
                                                      p=128))
            bqs = []
            for q in range(4):
                psa = ppool.tile([128, 512], mybir.dt.float32, tag="ps",
                                 name=f"psA_{rep}_{b}_{c}_{q}")
                for t in range(4):
                    nc.tensor.matmul(
                        psa[:, t * 128:(t + 1) * 128],
                        lhsT=xt[:, t, q * 128:(q + 1) * 128],
                        rhs=bd1_s[:], start=True, stop=True)
                bq = bpool.tile([128, 512], mybir.dt.float32, tag="bq",
                                name=f"bq_{rep}_{b}_{c}_{q}")
                dst = bq.rearrange("p (a t g) -> p t a g", a=8, t=4)
                src = psa.rearrange("p (t a g) -> p t a g", t=4, a=8)
                if ncopy % 2 == 0:
                    nc.vector.tensor_copy(dst, src)
                else:
                    nc.scalar.copy(dst, src)
                ncopy += 1
                bqs.append(bq)
            bqs_c.append(bqs)
        for h2 in range(2):
            st = fpool.tile([128, 6144], mybir.dt.float32, tag="st",
                            name=f"st_{rep}_{b}_{h2}")
            # [p, w2, h1, w1, h0, w0, c, q, xbl]
            sv = st.rearrange(
                "p (w2 h1 w1 h0 w0 c q z) -> p w2 h1 w1 h0 w0 c q z",
                w2=2, h1=2, w1=2, h0=2, w0=2, c=3, q=4)
            colt = "v5" in variant
            for c in range(3):
                for u in range(2 if colt else 4):
                    # v5: kl pair (2u, 2u+1) col-tiled into one [128, 512]
                    # psum: rows 0:64 = h0=0, 64:128 = h0=1 (h1 = u).
                    if colt:
                        psb = ppoolb.tile([128, 512], mybir.dt.float32,
                                          tag="psb",
                                          name=f"psB_{rep}_{b}_{h2}_{c}_{u}")
                        for q in range(4):
                            for h0 in range(2):
                                kH = 4 * h2 + 2 * u + h0
                                nc.tensor.matmul(
                                    psb[h0 * 64:(h0 + 1) * 64,
                                        q * 128:(q + 1) * 128],
                                    lhsT=bqs_c[c][q][:, kH * 64:(kH + 1) * 64],
                                    rhs=bd2_s[:], start=True, stop=True)
                        pv = psb.rearrange(
                            "p (q w2 w1 w0 z) -> p q w2 w1 w0 z",
                            q=4, w2=2, w1=2, w0=2)
                        for h0 in range(2):
                            for w2 in range(2):
                                for w1 in range(2):
                                    src = pv[h0 * 64:(h0 + 1) * 64,
                                             :, w2, w1, :, :]
                                    dst = sv[:64, w2, u, w1, h0, :, c, :, :] \
                                        .transpose([0, 2, 1, 3])
                                    if ncopy % 2 == 0:
                                        nc.vector.tensor_copy(dst, src)
                                    else:
                                        nc.scalar.copy(dst, src)
                                    ncopy += 1
                        continue
                    kl = u
                    h1, h0 = kl // 2, kl % 2
                    kH = 4 * h2 + kl
                    psb = ppoolb.tile([64, 512], mybir.dt.float32,
                                      tag="psb",
                                      name=f"psB_{rep}_{b}_{h2}_{c}_{kl}")
                    for q in range(4):
                        nc.tensor.matmul(
                            psb[:, q * 128:(q + 1) * 128],
                            lhsT=bqs_c[c][q][:, kH * 64:(kH + 1) * 64],
                            rhs=bd2_s[:], start=True, stop=True)
                    # psb f = (q, w2, w1, w0, xbl); copy per (w2, w1):
                    pv = psb.rearrange(
                        "p (q w2 w1 w0 z) -> p q w2 w1 w0 z",
                        q=4, w2=2, w1=2, w0=2)
                    for w2 in range(2):
                        for w1 in range(2):
                            src = pv[:, :, w2, w1, :, :]  # (p, q, w0, z)
                            dst = sv[:64, w2, h1, w1, h0, :, c, :, :] \
                                .transpose([0, 2, 1, 3])  # (p, q, w0, z)
                            if ncopy % 2 == 0:
                                nc.vector.tensor_copy(dst, src)
                            else:
                                nc.scalar.copy(dst, src)
                            ncopy += 1
            if do_out:
                dma_dst = out_v[b][96 * h2:96 * (h2 + 1)].transpose([1, 0, 2])
                nc.sync.dma_start(dma_dst, st[:64, :])


def _build_body(nc, variant, x_in, ov, bd1_s, bd2_s,
                xpool, bpool, fpool, ppool, mybir, rep=0):
    do_mm = "dmaonly" not in variant
    do_out = "noout" not in variant
    ndma = 0
    for img in range(12):
        xt = xpool.tile([128, 4, 512], mybir.dt.float32, tag="x",
                        name=f"x_{rep}_{img}")
        nc.gpsimd.dma_start(
            xt[:], x_in.ap()[img].rearrange("(t p) w -> p t w", p=128))

        tmax = 1 if "mm1of4" in variant else 4
        bqs = []
        if do_mm:
            for q in range(4):
                psa = ppool.tile([128, 512], mybir.dt.float32, tag="ps",
                                 name=f"psA_{rep}_{img}_{q}")
                for t in range(tmax):
                    nc.tensor.matmul(psa[:, t * 128:(t + 1) * 128],
                                     lhsT=xt[:, t, q * 128:(q + 1) * 128],
                                     rhs=bd1_s[:], start=True, stop=True)
                bq = bpool.tile([128, 512], mybir.dt.float32, tag="bq",
                                name=f"bq_{rep}_{img}_{q}")
                dst = bq.rearrange("p (a t g) -> p t a g", a=8, t=4)
                src = psa.rearrange("p (t a g) -> p t a g", t=4, a=8)
                nc.vector.tensor_copy(dst, src)
                bqs.append(bq)

        b, c = img // 3, img % 3
        for r in range(4):
            h2, h1 = r // 2, r % 2
            fr = fpool.tile([128, 512], mybir.dt.float32, tag="fr",
                            name=f"fr_{rep}_{img}_{r}")
            if do_mm:
                psb = ppool.tile([128, 512], mybir.dt.float32, tag="ps",
                                 name=f"psB_{rep}_{img}_{r}")
                for q in range(tmax):
                    nc.tensor.matmul(psb[:, q * 128:(q + 1) * 128],
                                     lhsT=bqs[q][:, r * 128:(r + 1) * 128],
                                     rhs=bd2_s[:], start=True, stop=True)
                dst = fr.rearrange("p (a q g) -> p q a g", a=8, q=4)
                src = psb.rearrange("p (q a g) -> p q a g", q=4, a=8)
                nc.vector.tensor_copy(dst, src)
            else:
                nc.vector.tensor_copy(fr[:], xt[:, r, :])

            if do_out:
                for kw in range(8):
                    w2, w1, w0 = kw // 4, (kw // 2) % 2, kw % 2
                    # dst dims (h0, yb, xb) matches src enumeration
                    dma_dst = ov[b, h2, w2, h1, w1, :, w0, c, :, :]
                    eng = nc.sync if ndma % 2 == 0 else nc.scalar
                    eng.dma_start(dma_dst, fr[:, kw * 64:(kw + 1) * 64])
                    ndma += 1


def _build_nc(variant="full"):
    from contextlib import ExitStack

    import concourse.tile as tile
    from concourse import bacc, mybir

    nc = bacc.Bacc("TRN2", target_bir_lowering=False, debug=False)

    x_in = nc.dram_tensor("x", [12, 512, 512], mybir.dt.float32,
                          kind="ExternalInput")
    bd1_d = nc.dram_tensor("bd1", [128, 128], mybir.dt.float32,
                           kind="ExternalInput")
    bd2_d = nc.dram_tensor("bd2", [128, 128], mybir.dt.float32,
                           kind="ExternalInput")
    out_d = nc.dram_tensor("out", [4, 192, 64, 64], mybir.dt.float32,
                           kind="ExternalOutput")
    # out view: [b, h2, w2, h1, w1, h0, w0, c, yb, xb]
    ov = out_d.ap().rearrange(
        "bb (h2 w2 h1 w1 h0 w0 c) yb xb -> bb h2 w2 h1 w1 h0 w0 c yb xb",
        h2=2, w2=2, h1=2, w1=2, h0=2, w0=2, c=3)

    v4 = "v4" in variant or "v5" in variant
    with tile.TileContext(nc) as tc, ExitStack() as ctx:
        cpool = ctx.enter_context(tc.tile_pool(name="consts", bufs=1))
        xpool = ctx.enter_context(tc.tile_pool(name="xin", bufs=4 if v4 else 2))
        bpool = ctx.enter_context(
            tc.tile_pool(name="bq", bufs=14 if v4 else 9))
        fpool = ctx.enter_context(
            tc.tile_pool(name="fr", bufs=3 if v4 else 9))
        ppool = ctx.enter_context(
            tc.tile_pool(name="ps", bufs=4 if v4 else 6, space="PSUM"))
        ppoolb = (ctx.enter_context(
            tc.tile_pool(name="psb", bufs=4, space="PSUM")) if v4 else None)

        bd1_s = cpool.tile([128, 128], mybir.dt.float32, tag="bd1")
        bd2_s = cpool.tile([128, 128], mybir.dt.float32, tag="bd2")
        nc.gpsimd.dma_start(bd1_s[:], bd1_d.ap())
        nc.gpsimd.dma_start(bd2_s[:], bd2_d.ap())

        if variant == "nop":
            nc.sync.dma_start(
                out_d.ap()[0, 0], bd1_s[:64, :64])
        else:
            reps = 1
            if variant == "double":
                reps = 2
            elif variant.startswith("rep"):
                reps = int(variant[3:].split("_")[0].replace("rep", "") or 1)
            outs_d = [out_d]
            for rep in range(1, reps):
                outs_d.append(nc.dram_tensor(
                    f"scr{rep}", [4, 192, 64, 64], mybir.dt.float32))
            for rep in range(reps):
                if v4:
                    _build_body_v4(nc, variant, x_in, outs_d[rep].ap(),
                                   bd1_s, bd2_s, xpool, bpool, fpool,
                                   ppool, ppoolb, mybir, rep=rep)
                else:
                    ovr = outs_d[rep].ap().rearrange(
                        "bb (h2 w2 h1 w1 h0 w0 c) yb xb -> "
                        "bb h2 w2 h1 w1 h0 w0 c yb xb",
                        h2=2, w2=2, h1=2, w1=2, h0=2, w0=2, c=3)
                    _build_body(nc, variant, x_in, ovr, bd1_s, bd2_s,
                                xpool, bpool, fpool, ppool, mybir, rep=rep)

    nc.compile()
    return nc


def _get_nc(variant="v5"):
    if variant not in _CACHE:
        _CACHE[variant] = _build_nc(variant)
    return _CACHE[variant]


def kernel(x: np.ndarray) -> np.ndarray:
    from concourse.bass_utils import run_bass_kernel_spmd

    assert x.shape == (32, 3, 512, 512) and x.dtype == np.float32
    nc = _get_nc()
    consts = _constants()
    in_maps = []
    for i in range(8):
        shard = np.ascontiguousarray(
            x[4 * i:4 * i + 4].reshape(12, 512, 512))
        in_maps.append({"x": shard, **consts})
    res = run_bass_kernel_spmd(nc, in_maps, core_ids=list(range(8)))
    return np.concatenate([r["out"] for r in res.results], axis=0)


# revision 38
# speedup vs baseline: 15.5856x; 1.2698x over previous
"""BlockDWT2d (3-level Haar wavelet packet, 8x8 blocks) on 8 TRN2 NeuronCores.

Input  x: (32, 3, 512, 512) fp32 -> output (32, 192, 64, 64) fp32.

Math: the 3-level full packet transform is separable: for each 8x8 input
block, out2d = (H8/8) @ X8 @ H8^T where H8 is the natural-binary-order
Hadamard matrix; output channel k_sub bit-interleaves the row/col
transform indices (kH, kW): k_sub = 32h2+16w2+8h1+4w1+2h0+w0, and the
full channel index is K = 3*k_sub + c.

Per-core pipeline (batch-sharded 4 samples/core, 12 images of 512x512):
  DMA in:  X [p=h(128) x4 tiles, f=w(512)] (contiguous, GPSIMD SWDGE)
  Pass A   (per q: 4 mms t=0..3): psA[:, 128t:] = X[:,t,128q:]^T @ BD1
           BD1[(g,i),(kH,g')] = H8[kH,i]/8 * delta_gg'
           -> [p=(xbl,j), f=(t,kH,g)]; one copy/regroup -> Bq[p,(kH,yb)]
  Pass B   (per r: 4 mms q=0..3): psB[:, 128q:] = Bq[:, 128r:]^T @ BD2
           BD2[(xbl,j),(kW,xbl')] = H8[kW,j] * delta_xbl
           -> [p=(h0,yb), f=(q,kW,xbl)]; one copy/regroup -> Fr[p,(kW,xb)]
  DMA out: direct HWDGE DMAs, one per (img, r, kW): src [128p, 64f],
           DRAM dims (h0, yb, xb) — 3-dim AP, 256B runs. Alternated
           between the SP and ACT HWDGE rings.
"""

import numpy as np

_CACHE = {}


def _h8():
    x = np.eye(8, dtype=np.float32).reshape(1, 8, 8)
    for _ in range(3):
        a, b = x[:, 0::2, :], x[:, 1::2, :]
        x = np.concatenate([a + b, a - b], axis=0)
    return x[:, 0, :]  # H8[k, i], entries +-1


def _interleave(kH, kW):
    h2, h1, h0 = (kH >> 2) & 1, (kH >> 1) & 1, kH & 1
    w2, w1, w0 = (kW >> 2) & 1, (kW >> 1) & 1, kW & 1
    return 32 * h2 + 16 * w2 + 8 * h1 + 4 * w1 + 2 * h0 + 1 * w0


def _constants():
    H8 = _h8()
    bd1 = np.zeros((128, 128), np.float32)  # [(g,i), (kH,g')]
    for g in range(16):
        bd1[g * 8:(g + 1) * 8, :].reshape(8, 8, 16)[:, :, g] = (H8.T / 8.0)
    bd2 = np.zeros((128, 128), np.float32)  # [(xbl,j), (kW,xbl')]
    for xbl in range(16):
        bd2[xbl * 8:(xbl + 1) * 8, :].reshape(8, 8, 16)[:, :, xbl] = H8.T
    return {"bd1": bd1, "bd2": bd2}


def _build_body_v4(nc, variant, x_in, out_v, bd1_s, bd2_s,
                   xpool, bpool, fpool, ppool, ppoolb, mybir, rep=0):
    """Pass-B M=64 (p=yb only); stage whole K-halves; 2 out-DMAs per sample.

    out_v: [4, 192, 64, 64] AP (ExternalOutput or scratch).
    Staging S_{b,h2} [64p(yb), f=(Klocal 96, xb 64)]; DMA dims
    [yb][K][xb]. Copy dst via 8-dim rearrange view.
    """
    do_out = "noout" not in variant
    in_eng = nc.sync if "insync" in variant else nc.gpsimd
    ncopy = 0
    for b in range(4):
        bqs_c = []
        for c in range(3):
            xt = xpool.tile([128, 4, 512], mybir.dt.float32, tag="x",
                            name=f"x_{rep}_{b}_{c}")
            in_eng.dma_start(
                xt[:], x_in.ap()[b * 3 + c].rearrange("(t p) w -> p t w",
                                                      p=128))
            bqs = []
            for q in range(4):
                psa = ppool.tile([128, 512], mybir.dt.float32, tag="ps",
                                 name=f"psA_{rep}_{b}_{c}_{q}")
                for t in range(4):
                    nc.tensor.matmul(
                        psa[:, t * 128:(t + 1) * 128],
                        lhsT=xt[:, t, q * 128:(q + 1) * 128],
                        rhs=bd1_s[:], start=True, stop=True)
                bq = bpool.tile([128, 512], mybir.dt.float32, tag="bq",
                                name=f"bq_{rep}_{b}_{c}_{q}")
                dst = bq.rearrange("p (a t g) -> p t a g", a=8, t=4)
                src = psa.rearrange("p (t a g) -> p t a g", t=4, a=8)
                if ncopy % 2 == 0:
                    nc.vector.tensor_copy(dst, src)
                else:
                    nc.scalar.copy(dst, src)
                ncopy += 1
                bqs.append(bq)
            bqs_c.append(bqs)
        for h2 in range(2):
            st = fpool.tile([128, 6144], mybir.dt.float32, tag="st",
                            name=f"st_{rep}_{b}_{h2}")
            # [p, w2, h1, w1, h0, w0, c, q, xbl]
            sv = st.rearrange(
                "p (w2 h1 w1 h0 w0 c q z) -> p w2 h1 w1 h0 w0 c q z",
                w2=2, h1=2, w1=2, h0=2, w0=2, c=3, q=4)
            colt = "v5" in variant
            for c in range(3):
                for u in range(2 if colt else 4):
                    # v5: kl pair (2u, 2u+1) col-tiled into one [128, 512]
                    # psum: rows 0:64 = h0=0, 64:128 = h0=1 (h1 = u).
                    if colt:
                        psb = ppoolb.tile([128, 512], mybir.dt.float32,
                                          tag="psb",
                                          name=f"psB_{rep}_{b}_{h2}_{c}_{u}")
                        for q in range(4):
                            for h0 in range(2):
                                kH = 4 * h2 + 2 * u + h0
                                nc.tensor.matmul(
                                    psb[h0 * 64:(h0 + 1) * 64,
                                        q * 128:(q + 1) * 128],
                                    lhsT=bqs_c[c][q][:, kH * 64:(kH + 1) * 64],
                                    rhs=bd2_s[:], start=True, stop=True)
                        pv = psb.rearrange(
                            "p (q w2 w1 w0 z) -> p q w2 w1 w0 z",
                            q=4, w2=2, w1=2, w0=2)
                        for h0 in range(2):
                            for w2 in range(2):
                                for w1 in range(2):
                                    src = pv[h0 * 64:(h0 + 1) * 64,
                                             :, w2, w1, :, :]
                                    dst = sv[:64, w2, u, w1, h0, :, c, :, :] \
                                        .transpose([0, 2, 1, 3])
                                    if ncopy % 2 == 0:
                                        nc.vector.tensor_copy(dst, src)
                                    else:
                                        nc.scalar.copy(dst, src)
                                    ncopy += 1
                        continue
                    kl = u
                    h1, h0 = kl // 2, kl % 2
                    kH = 4 * h2 + kl
                    psb = ppoolb.tile([64, 512], mybir.dt.float32,
                                      tag="psb",
                                      name=f"psB_{rep}_{b}_{h2}_{c}_{kl}")
                    for q in range(4):
                        nc.tensor.matmul(
                            psb[:, q * 128:(q + 1) * 128],
                            lhsT=bqs_c[c][q][:, kH * 64:(kH + 1) * 64],
                            rhs=bd2_s[:], start=True, stop=True)
                    # psb f = (q, w2, w1, w0, xbl); copy per (w2, w1):
                    pv = psb.rearrange(
                        "p (q w2 w1 w0 z) -> p q w2 w1 w0 z",
                        q=4, w2=2, w1=2, w0=2)
                    for w2 in range(2):
                        for w1 in range(2):
                            src = pv[:, :, w2, w1, :, :]  # (p, q, w0, z)
                            dst = sv[:64, w2, h1, w1, h0, :, c, :, :] \
                                .transpose([0, 2, 1, 3])  # (p, q, w0, z)
                            if ncopy % 2 == 0:
                                nc.vector.tensor_copy(dst, src)
                            else:
                                nc.scalar.copy(dst, src)
                            ncopy += 1
            if do_out:
                dma_dst = out_v[b][96 * h2:96 * (h2 + 1)].transpose([1, 0, 2])
                nc.sync.dma_start(dma_dst, st[:64, :])


def _build_body(nc, variant, x_in, ov, bd1_s, bd2_s,
                xpool, bpool, fpool, ppool, mybir, rep=0):
    do_mm = "dmaonly" not in variant
    do_out = "noout" not in variant
    ndma = 0
    for img in range(12):
        xt = xpool.tile([128, 4, 512], mybir.dt.float32, tag="x",
                        name=f"x_{rep}_{img}")
        nc.gpsimd.dma_start(
            xt[:], x_in.ap()[img].rearrange("(t p) w -> p t w", p=128))

        tmax = 1 if "mm1of4" in variant else 4
        bqs = []
        if do_mm:
            for q in range(4):
                psa = ppool.tile([128, 512], mybir.dt.float32, tag="ps",
                                 name=f"psA_{rep}_{img}_{q}")
                for t in range(tmax):
                    nc.tensor.matmul(psa[:, t * 128:(t + 1) * 128],
                                     lhsT=xt[:, t, q * 128:(q + 1) * 128],
                                     rhs=bd1_s[:], start=True, stop=True)
                bq = bpool.tile([128, 512], mybir.dt.float32, tag="bq",
                                name=f"bq_{rep}_{img}_{q}")
                dst = bq.rearrange("p (a t g) -> p t a g", a=8, t=4)
                src = psa.rearrange("p (t a g) -> p t a g", t=4, a=8)
                nc.vector.tensor_copy(dst, src)
                bqs.append(bq)

        b, c = img // 3, img % 3
        for r in range(4):
            h2, h1 = r // 2, r % 2
            fr = fpool.tile([128, 512], mybir.dt.float32, tag="fr",
                            name=f"fr_{rep}_{img}_{r}")
            if do_mm:
                psb = ppool.tile([128, 512], mybir.dt.float32, tag="ps",
                                 name=f"psB_{rep}_{img}_{r}")
                for q in range(tmax):
                    nc.tensor.matmul(psb[:, q * 128:(q + 1) * 128],
                                     lhsT=bqs[q][:, r * 128:(r + 1) * 128],
                                     rhs=bd2_s[:], start=True, stop=True)
                dst = fr.rearrange("p (a q g) -> p q a g", a=8, q=4)
                src = psb.rearrange("p (q a g) -> p q a g", q=4, a=8)
                nc.vector.tensor_copy(dst, src)
            else:
                nc.vector.tensor_copy(fr[:], xt[:, r, :])

            if do_out:
                for kw in range(8):
                    w2, w1, w0 = kw // 4, (kw // 2) % 2, kw % 2
                    # dst dims (h0, yb, xb) matches src enumeration
                    dma_dst = ov[b, h2, w2, h1, w1, :, w0, c, :, :]
                    eng = nc.sync if ndma % 2 == 0 else nc.scalar
                    eng.dma_start(dma_dst, fr[:, kw * 64:(kw + 1) * 64])
                    ndma += 1


def _build_nc(variant="full"):
    from contextlib import ExitStack

    import concourse.tile as tile
    from concourse import bacc, mybir

    nc = bacc.Bacc("TRN2", target_bir_lowering=False, debug=False)

    x_in = nc.dram_tensor("x", [12, 512, 512], mybir.dt.float32,
                          kind="ExternalInput")
    bd1_d = nc.dram_tensor("bd1", [128, 128], mybir.dt.float32,
                           kind="ExternalInput")
    bd2_d = nc.dram_tensor("bd2", [128, 128], mybir.dt.float32,
                           kind="ExternalInput")
    out_d = nc.dram_tensor("out", [4, 192, 64, 64], mybir.dt.float32,
                           kind="ExternalOutput")
    # out view: [b, h2, w2, h1, w1, h0, w0, c, yb, xb]
    ov = out_d.ap().rearrange(
        "bb (h2 w2 h1 w1 h0 w0 c) yb xb -> bb h2 w2 h1 w1 h0 w0 c yb xb",
        h2=2, w2=2, h1=2, w1=2, h0=2, w0=2, c=3)

    v4 = "v4" in variant or "v5" in variant
    with tile.TileContext(nc) as tc, ExitStack() as ctx:
        cpool = ctx.enter_context(tc.tile_pool(name="consts", bufs=1))
        xpool = ctx.enter_context(tc.tile_pool(name="xin", bufs=4 if v4 else 2))
        bpool = ctx.enter_context(
            tc.tile_pool(name="bq", bufs=14 if v4 else 9))
        fpool = ctx.enter_context(
            tc.tile_pool(name="fr", bufs=3 if v4 else 9))
        ppool = ctx.enter_context(
            tc.tile_pool(name="ps", bufs=4 if v4 else 6, space="PSUM"))
        ppoolb = (ctx.enter_context(
            tc.tile_pool(name="psb", bufs=4, space="PSUM")) if v4 else None)

        bd1_s = cpool.tile([128, 128], mybir.dt.float32, tag="bd1")
        bd2_s = cpool.tile([128, 128], mybir.dt.float32, tag="bd2")
        nc.gpsimd.dma_start(bd1_s[:], bd1_d.ap())
        nc.gpsimd.dma_start(bd2_s[:], bd2_d.ap())

        if variant == "nop":
            nc.sync.dma_start(
                out_d.ap()[0, 0], bd1_s[:64, :64])
        else:
            reps = 1
            if variant == "double":
                reps = 2
            elif variant.startswith("rep"):
                reps = int(variant[3:].split("_")[0].replace("rep", "") or 1)
            outs_d = [out_d]
            for rep in range(1, reps):
                outs_d.append(nc.dram_tensor(
                    f"scr{rep}", [4, 192, 64, 64], mybir.dt.float32))
            for rep in range(reps):
                if v4:
                    _build_body_v4(nc, variant, x_in, outs_d[rep].ap(),
                                   bd1_s, bd2_s, xpool, bpool, fpool,
                                   ppool, ppoolb, mybir, rep=rep)
                else:
                    ovr = outs_d[rep].ap().rearrange(
                        "bb (h2 w2 h1 w1 h0 w0 c) yb xb -> "
                        "bb h2 w2 h1 w1 h0 w0 c yb xb",
                        h2=2, w2=2, h1=2, w1=2, h0=2, w0=2, c=3)
                    _build_body(nc, variant, x_in, ovr, bd1_s, bd2_s,
                                xpool, bpool, fpool, ppool, mybir, rep=rep)

    nc.compile()
    return nc


def _get_nc(variant="v5"):
    if variant not in _CACHE:
        _CACHE[variant] = _build_nc(variant)
    return _CACHE[variant]


def kernel(x: np.ndarray) -> np.ndarray:
    from concourse.bass_utils import run_bass_kernel_spmd

    x = np.asarray(x, dtype=np.float32)
    assert x.shape == (32, 3, 512, 512)
    nc = _get_nc()
    consts = _constants()
    in_maps = []
    for i in range(8):
        shard = np.ascontiguousarray(
            x[4 * i:4 * i + 4].reshape(12, 512, 512))
        in_maps.append({"x": shard, **consts})
    res = run_bass_kernel_spmd(nc, in_maps, core_ids=list(range(8)))
    return np.concatenate([r["out"] for r in res.results], axis=0)
